# revision 1
# baseline (speedup 1.0000x reference)
"""CRD loss kernel for 8 Trainium2 NeuronCores.

Math notes (derived from the CRDLoss reference):
  - neg_scores gathers student rows idx[i,j] = j + (j>=i) which only ever
    touches student rows 0..10 ("head"); the rest of the student projection
    (and all logits / contrast_idx / idx inputs) are dead.
  - scores[i, :] for i>=11 is just anchor[i] @ s_head[0:10].T, a matmul.
    Rows 0..10 (on the shard owning them) need a shifted-head correction.
  - sum(log_D1)+sum(log_D0) = sum_i s_pos_i/T + 9*N*log(m*Pn)
                              - sum_{i,j} log(exp(s_ij/T) + m*Pn + EPS)
    so each core only returns per-(block,j) log-sums and pos-score sums; the
    host combines the 8 cores' partials into the two scalar losses.

Device layout per core (rows sharded 2048/core):
  - anchor features host-transposed to [128(k_in), 8(kt), 2048(r)] bf16;
    projections y^T accumulate in PSUM [128,512] per 512-row block.
  - the four row-blocks' score/norm matmuls are column-tiled (tile_position
    (0,32b)) into one [128,512] PSUM tile so the whole tail (rsqrt, scale,
    exp, log, reduce) runs as a few full-width ops per tensor.
  - 1/sqrt is computed as Exp(-0.5*Ln(x)) so ScalarE only ever needs the
    Exp/Ln tables (table reloads were the dominant cost of v1).
"""

import sys

for _p in ("/opt/trn_rl_repo", "/root/.axon_site/_ro/trn_rl_repo"):
    if _p not in sys.path:
        sys.path.insert(0, _p)

import math

import ml_dtypes
import numpy as np

import concourse.bass as bass  # noqa: F401
import concourse.tile as tile
from concourse import bacc, mybir
from concourse.bass_utils import run_bass_kernel_spmd

F32 = mybir.dt.float32
F32R = mybir.dt.float32r
BF16 = mybir.dt.bfloat16
FP8 = mybir.dt.float8e4
WSCALE = 64.0
AF = mybir.ActivationFunctionType

EPS = 1e-07
K = 10
T = 0.07
DIN = 1024
DOUT = 128
N = 16384
NCORES = 8
SH = N // NCORES          # 2048 rows per core
NKT = DIN // 128          # 8 k-tiles
BLK = 512
NBLK = SH // BLK          # 4 row blocks per core
NH = 16                   # head rows shipped (11 used)

# (anchor feature, anchor W, anchor b, side) per combo; side E=0 uses the
# entity student head, side R=1 the rel student head.
COMBOS = [
    ("entity_features_TeaE", "We_tE", "be_tE", 0),
    ("entity_features_TeaR", "We_tR", "be_tR", 0),
    ("rel_features_TeaE", "Wr_tE", "br_tE", 1),
    ("rel_features_TeaR", "Wr_tR", "br_tR", 1),
]
HEADS = [("entity_features_s", "We_s", "be_s"), ("rel_features_s", "Wr_s", "br_s")]

_CACHE = {}


def _build(c_const):
    """Build + compile the SPMD program. c_const = m*Pn + EPS baked into Ln."""
    nc = bacc.Bacc("TRN2", target_bir_lowering=False, debug=False)

    xdr = [nc.dram_tensor(f"x{q}", [128, NKT, SH], FP8, kind="ExternalInput")
           for q in range(4)]
    wdr = [nc.dram_tensor(f"w{q}", [128, NKT, DOUT], FP8, kind="ExternalInput")
           for q in range(4)]
    bdr = [nc.dram_tensor(f"b{q}", [DOUT, 1], F32, kind="ExternalInput")
           for q in range(4)]
    hdr = [nc.dram_tensor(f"h{s}", [128, NKT, NH], BF16, kind="ExternalInput")
           for s in range(2)]
    whdr = [nc.dram_tensor(f"wh{s}", [128, NKT, DOUT], BF16, kind="ExternalInput")
            for s in range(2)]
    bhdr = [nc.dram_tensor(f"bh{s}", [DOUT, 1], F32, kind="ExternalInput")
            for s in range(2)]
    mudr = nc.dram_tensor("mu", [NH, NH], F32, kind="ExternalInput")
    okdr = nc.dram_tensor("onk", [128, 32], BF16, kind="ExternalInput")
    okrdr = nc.dram_tensor("onkr", [128, NH], F32R, kind="ExternalInput")
    o1dr = nc.dram_tensor("on1", [1, 128], F32R, kind="ExternalInput")
    fldr = nc.dram_tensor("flag", [NH, 1], F32, kind="ExternalInput")
    outdr = nc.dram_tensor("out", [2, 128, 4], F32, kind="ExternalOutput")

    ln_invT = float(math.log(1.0 / T))

    with tile.TileContext(nc) as tc:
        with (
            tc.tile_pool(name="consts", bufs=1) as consts,
            tc.tile_pool(name="xp", bufs=3) as xp,
            tc.tile_pool(name="mid", bufs=3) as mid,
            tc.tile_pool(name="sco", bufs=2) as scop,
            tc.tile_pool(name="tiny", bufs=4) as tinyp,
            tc.tile_pool(name="pacc", bufs=2, space="PSUM") as pacc,
            tc.tile_pool(name="psco", bufs=2, space="PSUM") as psco,
            tc.tile_pool(name="pnsq", bufs=2, space="PSUM") as pnsq,
            tc.tile_pool(name="ptiny", bufs=2, space="PSUM") as ptiny,
        ):
            # ---- constants / small inputs ----
            w_t = [consts.tile([128, NKT, DOUT], FP8, name=f"w{q}", tag=f"w{q}")
                   for q in range(4)]
            b_t = [consts.tile([DOUT, 1], F32, name=f"b{q}", tag=f"b{q}")
                   for q in range(4)]
            h_t = [consts.tile([128, NKT, NH], BF16, name=f"h{s}", tag=f"h{s}")
                   for s in range(2)]
            wh_t = [consts.tile([128, NKT, DOUT], BF16, name=f"wh{s}", tag=f"wh{s}")
                    for s in range(2)]
            bh_t = [consts.tile([DOUT, 1], F32, name=f"bh{s}", tag=f"bh{s}")
                    for s in range(2)]
            mu_t = consts.tile([NH, NH], F32, tag="mu")
            fl_t = consts.tile([NH, 1], F32, tag="flag")
            ones_kn = consts.tile([128, 32], BF16, tag="ones_kn")
            ones_knr = consts.tile([128, NH], F32R, tag="ones_knr")
            ones_1p = consts.tile([1, 128], F32R, tag="ones_1p")
            acc_t = consts.tile([128, 4], F32, tag="acc")
            posa_t = consts.tile([128, 4], F32, tag="posacc")
            cb_t = consts.tile([128, 1], F32, tag="cb")
            lt_t = consts.tile([128, 1], F32, tag="lt")
            shead = [consts.tile([128, 32], BF16, name=f"shead{s}", tag=f"shead{s}")
                     for s in range(2)]
            sheadsh = [consts.tile([128, NH], BF16, name=f"sheadsh{s}",
                                   tag=f"sheadsh{s}") for s in range(2)]

            for q in range(4):
                nc.sync.dma_start(out=w_t[q][:], in_=wdr[q][:])
                nc.sync.dma_start(out=b_t[q][:], in_=bdr[q][:])
            for s in range(2):
                nc.sync.dma_start(out=h_t[s][:], in_=hdr[s][:])
                nc.sync.dma_start(out=wh_t[s][:], in_=whdr[s][:])
                nc.sync.dma_start(out=bh_t[s][:], in_=bhdr[s][:])
            nc.sync.dma_start(out=mu_t[:], in_=mudr[:])
            nc.sync.dma_start(out=fl_t[:], in_=fldr[:])
            nc.sync.dma_start(out=ones_kn[:], in_=okdr[:])
            nc.sync.dma_start(out=ones_knr[:], in_=okrdr[:])
            nc.sync.dma_start(out=ones_1p[:], in_=o1dr[:])
            x_t = [xp.tile([128, NKT, SH], FP8, name="xt", tag="x")
                   for _ in range(4)]
            for q in range(4):
                nc.sync.dma_start(out=x_t[q][:], in_=xdr[q][:])
            nc.vector.memset(acc_t[:], 0.0)
            nc.vector.memset(posa_t[:], 0.0)
            nc.vector.memset(cb_t[:], float(c_const))
            nc.vector.memset(lt_t[:], ln_invT)

            # ---- student heads: normalized s_head^T [128(dout), 32] ----
            yhs, lnhs, inv1s = [], [], []
            for s in range(2):
                yh_ps = ptiny.tile([128, 32], F32, name="yh_ps", tag="ptiny")
                for kt in range(NKT):
                    nc.tensor.matmul(
                        out=yh_ps[:, 0:NH],
                        lhsT=wh_t[s][:, kt, :],
                        rhs=h_t[s][:, kt, :],
                        start=(kt == 0),
                        stop=(kt == NKT - 1),
                    )
                yh = tinyp.tile([128, NH], F32, name=f"yh{s}", tag=f"yh{s}")
                nc.vector.tensor_scalar_add(out=yh[:], in0=yh_ps[:, 0:NH],
                                            scalar1=bh_t[s][:])
                sqh = tinyp.tile([128, NH], F32R, name="sqh", tag="sqh")
                nc.vector.tensor_mul(out=sqh[:], in0=yh[:], in1=yh[:])
                nsqh_ps = ptiny.tile([128, 32], F32, name="nsqh_ps", tag="ptiny")
                nc.tensor.matmul(
                    out=nsqh_ps[0:NH, 0:NH],
                    lhsT=ones_knr[:],
                    rhs=sqh[:],
                    start=True,
                    stop=True,
                )
                # 1/sqrt(v) = Exp(-0.5 * Ln(v)) — keeps ScalarE on Exp/Ln only
                lnh = tinyp.tile([1, NH], F32, name=f"lnh{s}", tag=f"lnh{s}")
                nc.scalar.activation(out=lnh[:], in_=nsqh_ps[0:1, 0:NH], func=AF.Ln)
                yhs.append(yh)
                lnhs.append(lnh)
            for s in range(2):
                inv1 = tinyp.tile([1, NH], F32R, name=f"inv1{s}", tag=f"inv1{s}")
                nc.scalar.activation(out=inv1[:], in_=lnhs[s][:], func=AF.Exp,
                                     scale=-0.5)
                inv1s.append(inv1)
            for s in range(2):
                invb_ps = ptiny.tile([128, 32], F32, name="invb_ps", tag="ptiny")
                nc.tensor.matmul(
                    out=invb_ps[0:128, 0:NH],
                    lhsT=ones_1p[:],
                    rhs=inv1s[s][:],
                    start=True,
                    stop=True,
                )
                nc.vector.tensor_mul(out=shead[s][:, 0:NH], in0=yhs[s][:],
                                     in1=invb_ps[:, 0:NH])
                # pad cols 16..31 (fills the 32-wide col-tile strip)
                nc.vector.tensor_copy(out=shead[s][:, NH:32],
                                      in_=shead[s][:, 0:NH])
                # shifted head (col j = head j+1) for the local-rows correction
                nc.vector.tensor_copy(out=sheadsh[s][:, 0:NH - 1],
                                      in_=shead[s][:, 1:NH])
                nc.vector.tensor_copy(out=sheadsh[s][:, NH - 1:NH],
                                      in_=shead[s][:, 0:1])

            # ---- main: 4 anchors, 4 col-tiled row-blocks each ----
            for q, (_, _, _, s) in enumerate(COMBOS):
                sco_ps = psco.tile([128, BLK], F32, tag="psco")
                nsq_ps = pnsq.tile([128, BLK], F32, tag="pnsq")
                s1_ps = ptiny.tile([128, 32], F32, name="s1_ps", tag="ptiny")
                for blk in range(NBLK):
                    r0 = blk * BLK
                    acc_ps = pacc.tile([128, BLK], F32, tag="pacc")
                    for kt in range(NKT):
                        nc.tensor.matmul(
                            out=acc_ps[:],
                            lhsT=w_t[q][:, kt, :],
                            rhs=x_t[q][:, kt, r0:r0 + BLK],
                            start=(kt == 0),
                            stop=(kt == NKT - 1),
                        )
                    yb = mid.tile([128, BLK], BF16, tag="yb")
                    nc.vector.tensor_scalar_add(out=yb[:], in0=acc_ps[:],
                                                scalar1=b_t[q][:])
                    sq = mid.tile([128, BLK], BF16, tag="sq")
                    nc.vector.tensor_mul(out=sq[:], in0=yb[:], in1=yb[:])
                    p0 = 32 * blk
                    nc.tensor.matmul(
                        out=sco_ps[p0:p0 + 32, :],
                        lhsT=shead[s][:],
                        rhs=yb[:],
                        start=True,
                        stop=True,
                        tile_position=(0, p0),
                    )
                    nc.tensor.matmul(
                        out=nsq_ps[p0:p0 + 32, :],
                        lhsT=ones_kn[:],
                        rhs=sq[:],
                        start=True,
                        stop=True,
                        tile_position=(0, p0),
                    )
                    if blk == 0:
                        # shifted-head scores for the local-rows correction
                        nc.tensor.matmul(
                            out=s1_ps[0:NH, 0:NH],
                            lhsT=sheadsh[s][:],
                            rhs=yb[:, 0:NH],
                            start=True,
                            stop=True,
                        )

                # tail for all 4 blocks at once, full 128-partition ops
                rln = scop.tile([128, BLK], F32, tag="rln")
                nc.scalar.activation(out=rln[:], in_=nsq_ps[:], func=AF.Ln)
                rsq = scop.tile([128, BLK], F32, tag="rsq")
                nc.scalar.activation(out=rsq[:], in_=rln[:], func=AF.Exp,
                                     scale=-0.5, bias=lt_t[:])
                st = scop.tile([128, BLK], F32, tag="st")
                nc.vector.tensor_mul(out=st[:], in0=sco_ps[:], in1=rsq[:])

                # local rows 0..10 shifted-head correction (flag gates core 0)
                d = tinyp.tile([NH, NH], F32, tag="d")
                nc.vector.tensor_mul(out=d[0:10, 0:11], in0=s1_ps[0:10, 0:11],
                                     in1=rsq[0:10, 0:11])
                nc.vector.tensor_sub(out=d[0:10, 0:11], in0=d[0:10, 0:11],
                                     in1=st[0:10, 0:11])
                nc.vector.tensor_mul(out=d[0:10, 0:11], in0=d[0:10, 0:11],
                                     in1=mu_t[0:10, 0:11])
                nc.vector.tensor_scalar_mul(out=d[0:10, 0:11],
                                            in0=d[0:10, 0:11],
                                            scalar1=fl_t[0:10, :])
                nc.vector.tensor_add(out=st[0:10, 0:11], in0=st[0:10, 0:11],
                                     in1=d[0:10, 0:11])

                ex = scop.tile([128, BLK], F32, tag="ex")
                nc.scalar.activation(out=ex[:], in_=st[:], func=AF.Exp)
                lacc = tinyp.tile([128, 1], F32, tag="lacc")
                nc.scalar.activation(out=ex[:], in_=ex[:], func=AF.Ln,
                                     bias=cb_t[:], accum_out=lacc[:])
                posr = tinyp.tile([128, 1], F32, tag="posr")
                nc.vector.reduce_sum(out=posr[:], in_=st[:],
                                     axis=mybir.AxisListType.X)
                nc.vector.tensor_add(out=acc_t[:, q:q + 1],
                                     in0=acc_t[:, q:q + 1], in1=lacc[:])
                nc.vector.tensor_add(out=posa_t[:, q:q + 1],
                                     in0=posa_t[:, q:q + 1], in1=posr[:])

            nc.sync.dma_start(out=outdr[0], in_=acc_t[:])
            nc.sync.dma_start(out=outdr[1], in_=posa_t[:])

    nc.compile()
    return nc


def _pack_x(feat):
    """[B,TS,DIN] f32 -> per-core [128, NKT, SH] fp8-e4m3, k-partition-major."""
    f = np.ascontiguousarray(np.asarray(feat, dtype=np.float32)).reshape(N, DIN)
    # (core, r, kt, p) -> (core, p, kt, r)
    v = f.reshape(NCORES, SH, NKT, 128).transpose(0, 3, 2, 1)
    return np.ascontiguousarray(v.astype(ml_dtypes.float8_e4m3))


def _pack_w(w):
    v = np.asarray(w, dtype=np.float32).reshape(NKT, 128, DOUT).transpose(1, 0, 2)
    return np.ascontiguousarray(v.astype(ml_dtypes.bfloat16))


def _pack_w8(w):
    # x64 lands typical N(0, 0.02^2) weights in the fp8 normal range; the scale
    # cancels in the L2 normalization (biases scaled to match).
    v = (np.asarray(w, dtype=np.float32) * WSCALE).reshape(NKT, 128, DOUT)
    return np.ascontiguousarray(v.transpose(1, 0, 2).astype(ml_dtypes.float8_e4m3))


def _pack_h(feat):
    f = np.asarray(feat, dtype=np.float32).reshape(N, DIN)[0:NH]  # [16, 1024]
    v = f.T.reshape(NKT, 128, NH).transpose(1, 0, 2)
    return np.ascontiguousarray(v.astype(ml_dtypes.bfloat16))


def kernel(**inputs):
    M = int(np.asarray(inputs["M"]))
    m = K - 1
    Pn = 1.0 / float(M)
    c_const = m * Pn + EPS

    key = ("v6", M)
    if key not in _CACHE:
        _CACHE[key] = _build(c_const)
    nc = _CACHE[key]

    xs = [_pack_x(inputs[COMBOS[q][0]]) for q in range(4)]
    ws = [_pack_w8(inputs[COMBOS[q][1]]) for q in range(4)]
    bs = [np.ascontiguousarray(
        np.asarray(inputs[COMBOS[q][2]], dtype=np.float32).reshape(DOUT, 1) * WSCALE)
        for q in range(4)]
    hs = [_pack_h(inputs[HEADS[s][0]]) for s in range(2)]
    whs = [_pack_w(inputs[HEADS[s][1]]) for s in range(2)]
    bhs = [np.ascontiguousarray(
        np.asarray(inputs[HEADS[s][2]], dtype=np.float32).reshape(DOUT, 1))
        for s in range(2)]

    j = np.arange(NH)[:, None]
    i = np.arange(NH)[None, :]
    mu = (j >= i).astype(np.float32)  # 1 where the shifted head row is used

    in_maps = []
    for cid in range(NCORES):
        im = {}
        for q in range(4):
            im[f"x{q}"] = xs[q][cid]
            im[f"w{q}"] = ws[q]
            im[f"b{q}"] = bs[q]
        for s in range(2):
            im[f"h{s}"] = hs[s]
            im[f"wh{s}"] = whs[s]
            im[f"bh{s}"] = bhs[s]
        im["mu"] = mu
        im["onk"] = np.ones((128, 32), ml_dtypes.bfloat16)
        im["onkr"] = np.ones((128, NH), np.float32)
        im["on1"] = np.ones((1, 128), np.float32)
        im["flag"] = np.full((NH, 1), 1.0 if cid == 0 else 0.0, dtype=np.float32)
        in_maps.append(im)

    res = run_bass_kernel_spmd(nc, in_maps, list(range(NCORES)))
    global LAST_RESULT
    LAST_RESULT = res

    outs = np.stack([np.asarray(res.results[cid]["out"])
                     for cid in range(NCORES)])  # [8, 2, 128, 4]
    rows_log = np.concatenate([32 * b + np.arange(10) for b in range(NBLK)])
    rows_pos = np.array([32 * b for b in range(NBLK)])
    slog = outs[:, 0, rows_log, :].sum(axis=(0, 1))    # [4]
    spos_T = outs[:, 1, rows_pos, :].sum(axis=(0, 1))  # [4], already / T
    const = 9.0 * N * np.log(m * Pn)
    loss = -(spos_T + const - slog) / N                # [4]
    return np.array([loss[0] + loss[1], loss[2] + loss[3]], dtype=np.float32)


if __name__ == "__main__":
    rng = np.random.default_rng(0)
    fake = {}
    for nm in ("entity_features_s", "rel_features_s", "entity_features_TeaE",
               "rel_features_TeaE", "entity_features_TeaR", "rel_features_TeaR"):
        fake[nm] = rng.standard_normal((16, 1024, DIN), dtype=np.float32)
    for nm in ("entity_logits_TeaE", "rel_logits_TeaE", "entity_logits_TeaR",
               "rel_logits_TeaR"):
        fake[nm] = rng.standard_normal((16, 1024, 100), dtype=np.float32)
    for pn in ("We_s", "We_tE", "We_tR", "Wr_s", "Wr_tE", "Wr_tR"):
        fake[pn] = (rng.standard_normal((DIN, DOUT), dtype=np.float32) * 0.02)
        fake[pn.replace("W", "b", 1)] = np.zeros((DOUT,), np.float32)
    fake["contrast_idx"] = rng.integers(0, 50000, size=(N,))
    fake["idx"] = rng.integers(0, 50000, size=(N,))
    fake["M"] = 50000
    print(kernel(**fake))



# revision 24
# speedup vs baseline: 1.4840x; 1.4840x over previous
"""CRD loss kernel for 8 Trainium2 NeuronCores (v8).

Math notes (derived from the CRDLoss reference):
  - neg_scores gathers student rows idx[i,j] = j + (j>=i) which only ever
    touches student rows 0..10 ("head"); the rest of the student projection
    (and all logits / contrast_idx / idx inputs) are dead.
  - log(exp(u)+c) = u + c*exp(-u) + O(c^2 e^-2u) with c = m/M + EPS ~ 1.8e-4
    and u = s/T in ~N(0, 1.26^2): the quadratic term contributes ~3e-8
    relative loss error.  So slog = sum(u) (plain DVE reduce) + c*sum(e^-u)
    (one Exp activation with the hw accumulator).  Scalar only ever runs
    Exp/Identity -> exactly one ACT_TABLE_LOAD, warmed at kernel start.
  - 1/sqrt on the Vector engine: 0x5f3759df bit hack (in f32 value space,
    no int ops) + one Newton iteration (~0.2% worst case, ~1e-5 loss err).

Layout per core (rows sharded 2048/core):
  - anchor features x: [128(k), combo, block, kt, 512(r)] fp8; combo 0
    lands in 512-row block chunks, combos 1..3 whole, all on the sync
    queue in consumption order.
  - projection y^T accumulates in PSUM [128,512] per block via fp8
    DoubleRow matmuls (2 k-tiles per instruction, 2x rate); bias-add +
    bf16 convert runs on Scalar (Identity, per-partition bias).
  - scores: per 128-anchor chunk, matmul lhsT=yb[:,chunk] (bf16) with
    rhs=[h_hat0..h_hat10] gives [128 anchors, 11] scores; lhsT=sq chunk
    with a ones column gives the norm^2 in col 11.  Anchors on partitions
    means rsq broadcasts with a stride-0 AP, the shifted-head correction
    (rows 0..10 of core 0) is a free AP column offset, and the reduce /
    accumulators directly yield the per-core partial sums.
  - emission order software-pipelines: proj(q0), heads, proj(q1),
    score(q0)+tail(q0), proj(q2), score(q1)+tail(q1), ...
"""

import sys

for _p in ("/opt/trn_rl_repo", "/root/.axon_site/_ro/trn_rl_repo"):
    if _p not in sys.path:
        sys.path.insert(0, _p)

import math
import os

import ml_dtypes
import numpy as np

import concourse.bass as bass  # noqa: F401
import concourse.tile as tile
from concourse import bacc, mybir
from concourse.bass_utils import run_bass_kernel_spmd

F32 = mybir.dt.float32
F32R = mybir.dt.float32r
BF16 = mybir.dt.bfloat16
FP8 = mybir.dt.float8e4
U32 = mybir.dt.uint32
WSCALE = 64.0
AF = mybir.ActivationFunctionType
ALU = mybir.AluOpType

EPS = 1e-07
K = 10
T = 0.07
DIN = 1024
DOUT = 128
N = 16384
NCORES = 8
SH = N // NCORES          # 2048 rows per core
NKT = DIN // 128          # 8 k-tiles
BLK = 512
NBLK = SH // BLK          # 4 row blocks per core
CH = 128                  # anchors per score chunk
NCHB = BLK // CH          # 4 chunks per block
NCH = SH // CH            # 16 chunks per combo
NHH = 11                  # head columns used (h_hat 0..10)
SC = NHH + 1              # psum cols per chunk (11 scores + 1 norm^2)
NH = 16                   # head rows shipped

# (anchor feature, anchor W, anchor b, side); side 0 = entity student head.
COMBOS = [
    ("entity_features_TeaE", "We_tE", "be_tE", 0),
    ("entity_features_TeaR", "We_tR", "be_tR", 0),
    ("rel_features_TeaE", "Wr_tE", "br_tE", 1),
    ("rel_features_TeaR", "Wr_tR", "br_tR", 1),
]
HEADS = [("entity_features_s", "We_s", "be_s"), ("rel_features_s", "Wr_s", "br_s")]

MAGIC = 0x5F3759DF
USE_DR = os.environ.get("K_DR", "1") == "1"
USE_GPS_MUL = os.environ.get("K_GPSMUL", "0") == "1"
USE_TTR = os.environ.get("K_TTR", "0") == "1"
YB_SCALAR = os.environ.get("K_YBSC", "1") == "1"

_CACHE = {}


def _newton_rsqrt(nc, pool, v, n, final_scale=1.0, p=128):
    """r = final_scale / sqrt(v) on DVE, no activation tables.

    One Newton iteration after the bit hack: <=0.18% rel error, which is
    ~1e-5 relative on the final loss (errors are random across anchors).
    """
    r0 = pool.tile([p, n], F32, tag="nw_r0")
    t = pool.tile([p, n], F32, tag="nw_t")
    r1 = pool.tile([p, n], F32, tag="nw_r1")
    # r0_bits = MAGIC - v_bits/2, in f32 value arithmetic (the +-bit
    # rounding is noise vs the hack's own error).  The dtype-mismatched
    # read/write does the u32<->f32 value conversion.
    nc.vector.tensor_scalar(
        out=t[:], in0=v.bitcast(U32),
        scalar1=-0.5, scalar2=float(MAGIC),
        op0=ALU.mult, op1=ALU.add,
    )
    nc.vector.tensor_scalar(
        out=r0.bitcast(U32), in0=t[:], scalar1=0.0, scalar2=None, op0=ALU.add,
    )
    # r1 = r0 * fs * (1.5 - 0.5 v r0^2)
    nc.vector.tensor_mul(out=t[:], in0=r0[:], in1=r0[:])
    nc.vector.tensor_mul(out=t[:], in0=t[:], in1=v[:])
    nc.vector.tensor_scalar(out=t[:], in0=t[:],
                            scalar1=-0.5 * final_scale,
                            scalar2=1.5 * final_scale,
                            op0=ALU.mult, op1=ALU.add)
    nc.vector.tensor_mul(out=r1[:], in0=r0[:], in1=t[:])
    return r1


def _build(c_const):
    """Build + compile the SPMD program. c_const = m*Pn + EPS."""
    nc = bacc.Bacc("TRN2", target_bir_lowering=False, debug=False)

    xdr = nc.dram_tensor("x", [128, 4, NBLK, NKT, BLK], FP8, kind="ExternalInput")
    wdr = nc.dram_tensor("w", [128, 4, NKT, DOUT], FP8, kind="ExternalInput")
    bdr = nc.dram_tensor("b", [DOUT, 4], F32, kind="ExternalInput")
    whdr = nc.dram_tensor("wh", [128, 2, NKT, DOUT], BF16, kind="ExternalInput")
    hdr = nc.dram_tensor("h", [128, 2, NKT, NH], BF16, kind="ExternalInput")
    bhdr = nc.dram_tensor("bh", [DOUT, 2], F32, kind="ExternalInput")
    fbdr = nc.dram_tensor("fb", [128, 20], F32, kind="ExternalInput")
    outdr = nc.dram_tensor("out", [128, 12], F32, kind="ExternalOutput")

    with tile.TileContext(nc) as tc:
        with (
            tc.tile_pool(name="consts", bufs=1) as consts,
            tc.tile_pool(name="xp", bufs=1) as xp,
            tc.tile_pool(name="ybp", bufs=2) as ybp,
            tc.tile_pool(name="sqp", bufs=2) as sqp,
            tc.tile_pool(name="stp", bufs=2) as stp,
            tc.tile_pool(name="tiny", bufs=4) as tinyp,
            tc.tile_pool(name="pacc", bufs=3, space="PSUM") as pacc,
            tc.tile_pool(name="psco", bufs=2, space="PSUM") as psco,
            tc.tile_pool(name="ptiny", bufs=2, space="PSUM") as ptiny,
        ):
            # ---- tiles ----
            x_t = xp.tile([128, 4, NBLK, NKT, BLK], FP8, tag="x")
            w_t = consts.tile([128, 4, NKT, DOUT], FP8, tag="w")
            b_t = consts.tile([DOUT, 4], F32, tag="b")
            wh_t = consts.tile([128, 2, NKT, DOUT], BF16, tag="wh")
            h_t = consts.tile([128, 2, NKT, NH], BF16, tag="h")
            bh_t = consts.tile([DOUT, 2], F32, tag="bh")
            fb_t = consts.tile([128, 20], F32, tag="fb")
            ones_knr = consts.tile([128, NH], F32, tag="ones_knr")
            ones_1p = consts.tile([1, 128], F32, tag="ones_1p")
            onebf = consts.tile([128, 1], F32, tag="onebf")
            hh = [consts.tile([128, 16], BF16, name=f"hh{s}", tag=f"hh{s}")
                  for s in range(2)]
            acc_t = consts.tile([128, 12], F32, tag="acc")

            # ---- DMA issue: x on sync (consumption order), consts on gpsimd
            for blk in range(NBLK):
                nc.sync.dma_start(out=x_t[:, 0, blk], in_=xdr[:, 0, blk])
            for q in range(1, 4):
                nc.sync.dma_start(out=x_t[:, q], in_=xdr[:, q])
            nc.gpsimd.dma_start(out=w_t[:], in_=wdr[:])
            nc.gpsimd.dma_start(out=wh_t[:], in_=whdr[:])
            nc.gpsimd.dma_start(out=h_t[:], in_=hdr[:])
            nc.gpsimd.dma_start(out=b_t[:], in_=bdr[:])
            nc.gpsimd.dma_start(out=bh_t[:], in_=bhdr[:])
            nc.gpsimd.dma_start(out=fb_t[:], in_=fbdr[:])

            nc.vector.memset(ones_knr[:], 1.0)
            nc.vector.memset(ones_1p[:], 1.0)
            nc.vector.memset(onebf[:], 1.0)

            # warm the exp table while DMAs stream
            spw = tinyp.tile([1, 1], F32, tag="spw")
            nc.vector.memset(spw[:], 0.0)
            nc.scalar.activation(out=spw[:], in_=spw[:], func=AF.Exp)

            yb_ts, sq_ts, sco_pss = {}, {}, {}

            def emit_proj(q):
                yb_t = ybp.tile([128, NBLK, BLK], BF16, tag="yb")
                sq_t = sqp.tile([128, NBLK, BLK], BF16, tag="sq")
                yb_ts[q], sq_ts[q] = yb_t, sq_t
                acc_list = []
                for blk in range(NBLK):
                    acc_ps = pacc.tile([128, BLK], F32, tag="pacc")
                    if USE_DR:
                        for kt in range(0, NKT, 2):
                            nc.tensor.matmul(
                                out=acc_ps[:],
                                lhsT=w_t[:, q, kt:kt + 2, :],
                                rhs=x_t[:, q, blk, kt:kt + 2, :],
                                start=(kt == 0),
                                stop=(kt == NKT - 2),
                                perf_mode=mybir.MatmulPerfMode.DoubleRow,
                            )
                    else:
                        for kt in range(NKT):
                            nc.tensor.matmul(
                                out=acc_ps[:],
                                lhsT=w_t[:, q, kt, :],
                                rhs=x_t[:, q, blk, kt, :],
                                start=(kt == 0),
                                stop=(kt == NKT - 1),
                            )
                    acc_list.append(acc_ps)
                sq_eng = nc.gpsimd if USE_GPS_MUL else nc.vector
                for blk in range(NBLK):
                    if YB_SCALAR:
                        nc.scalar.activation(out=yb_t[:, blk],
                                             in_=acc_list[blk][:],
                                             func=AF.Identity,
                                             bias=b_t[:, q:q + 1])
                    else:
                        nc.vector.tensor_scalar_add(out=yb_t[:, blk],
                                                    in0=acc_list[blk][:],
                                                    scalar1=b_t[:, q:q + 1])
                    sq_eng.tensor_mul(out=sq_t[:, blk], in0=yb_t[:, blk],
                                      in1=yb_t[:, blk])

            def emit_heads():
                for s in range(2):
                    yh_ps = ptiny.tile([128, NH], F32, name="yh_ps", tag="ptiny")
                    for kt in range(NKT):
                        nc.tensor.matmul(
                            out=yh_ps[:],
                            lhsT=wh_t[:, s, kt, :],
                            rhs=h_t[:, s, kt, :],
                            start=(kt == 0),
                            stop=(kt == NKT - 1),
                        )
                    yh = tinyp.tile([128, NH], F32, name=f"yh{s}", tag=f"yh{s}")
                    nc.vector.tensor_scalar_add(out=yh[:], in0=yh_ps[:],
                                                scalar1=bh_t[:, s:s + 1])
                    sqh = tinyp.tile([128, NH], F32R, name="sqh", tag="sqh")
                    nc.vector.tensor_mul(out=sqh[:], in0=yh[:], in1=yh[:])
                    nsq_ps = ptiny.tile([NH, NH], F32, name="nsqh_ps", tag="ptiny")
                    nc.tensor.matmul(out=nsq_ps[:],
                                     lhsT=ones_knr[:].bitcast(F32R),
                                     rhs=sqh[:], start=True, stop=True)
                    nsqh = tinyp.tile([1, NH], F32, name="nsqh", tag="nsqh")
                    nc.vector.tensor_copy(out=nsqh[:], in_=nsq_ps[0:1, :])
                    rsqh = _newton_rsqrt(nc, tinyp, nsqh, NH, p=1)
                    rsqh_r = tinyp.tile([1, NH], F32R, name="rsqh_r",
                                        tag="rsqh_r")
                    nc.vector.tensor_copy(out=rsqh_r[:], in_=rsqh[:])
                    rsqb_ps = ptiny.tile([128, NH], F32, name="rsqb_ps",
                                         tag="ptiny")
                    nc.tensor.matmul(out=rsqb_ps[:],
                                     lhsT=ones_1p[:].bitcast(F32R),
                                     rhs=rsqh_r[:], start=True, stop=True)
                    nc.vector.tensor_mul(out=hh[s][:, 0:NHH], in0=yh[:, 0:NHH],
                                         in1=rsqb_ps[:, 0:NHH])
                    nc.vector.tensor_copy(out=hh[s][:, NHH:NHH + 1],
                                          in_=onebf[:])

            def emit_score(q):
                s = COMBOS[q][3]
                yb_t, sq_t = yb_ts[q], sq_ts[q]
                sco_ps = psco.tile([128, NCH, SC], F32, tag="psco")
                sco_pss[q] = sco_ps
                for blk in range(NBLK):
                    for j in range(NCHB):
                        c = NCHB * blk + j
                        cs = slice(CH * j, CH * j + CH)
                        nc.tensor.matmul(
                            out=sco_ps[:, c, 0:NHH],
                            lhsT=yb_t[:, blk, cs],
                            rhs=hh[s][:, 0:NHH],
                            start=True, stop=True,
                        )
                        nc.tensor.matmul(
                            out=sco_ps[:, c, NHH:SC],
                            lhsT=sq_t[:, blk, cs],
                            rhs=hh[s][:, NHH:NHH + 1],
                            start=True, stop=True,
                        )

            def emit_tail(q):
                sco_ps = sco_pss[q]
                v = stp.tile([128, NCH], F32, tag="v")
                nc.vector.tensor_copy(out=v[:], in_=sco_ps[:, :, NHH])
                rsq = _newton_rsqrt(nc, stp, v, NCH, final_scale=1.0 / T)
                # st = u for cols 0..9 (and col 10 of chunk 0 for the shift)
                st_t = stp.tile([128, NCH, SC], F32, tag="st")
                if USE_TTR:
                    nc.vector.tensor_tensor_reduce(
                        out=st_t[:, :, 0:K],
                        in0=sco_ps[:, :, 0:K],
                        in1=rsq[:].unsqueeze(2).broadcast_to((128, NCH, K)),
                        scale=1.0, scalar=0.0,
                        op0=ALU.mult, op1=ALU.add,
                        accum_out=acc_t[:, 4 + q:5 + q],
                    )
                else:
                    nc.vector.tensor_mul(
                        out=st_t[:, :, 0:K],
                        in0=sco_ps[:, :, 0:K],
                        in1=rsq[:].unsqueeze(2).broadcast_to((128, NCH, K)),
                    )
                    ur = stp.tile([128, NCH], F32, tag="ur")
                    nc.vector.reduce_sum(out=ur[:], in_=st_t[:, :, 0:K],
                                         axis=mybir.AxisListType.X)
                    nc.vector.reduce_sum(out=acc_t[:, 4 + q:5 + q],
                                         in_=ur[:],
                                         axis=mybir.AxisListType.X)
                nc.vector.tensor_mul(out=st_t[:, 0, K:NHH],
                                     in0=sco_ps[:, 0, K:NHH],
                                     in1=rsq[:, 0:1])
                # local rows 0..9 shifted-head correction (flag gates core 0)
                d = tinyp.tile([16, 16], F32, tag="d")
                nc.vector.tensor_sub(out=d[0:10, 0:10],
                                     in0=st_t[0:10, 0, 1:11],
                                     in1=st_t[0:10, 0, 0:10])
                nc.vector.tensor_mul(out=d[0:10, 0:10], in0=d[0:10, 0:10],
                                     in1=fb_t[0:10, 0:10])
                nc.vector.tensor_scalar_mul(out=d[0:10, 0:10],
                                            in0=d[0:10, 0:10],
                                            scalar1=fb_t[0:10, 16:17])
                nc.vector.tensor_add(out=st_t[0:10, 0, 0:10],
                                     in0=st_t[0:10, 0, 0:10],
                                     in1=d[0:10, 0:10])
                dr = tinyp.tile([16, 1], F32, tag="dr")
                nc.vector.reduce_sum(out=dr[0:10, :], in_=d[0:10, 0:10],
                                     axis=mybir.AxisListType.X)
                nc.vector.tensor_add(out=acc_t[0:10, 4 + q:5 + q],
                                     in0=acc_t[0:10, 4 + q:5 + q],
                                     in1=dr[0:10, :])
                # c * sum(exp(-u)) correction term -> acc col q
                sp_scr = stp.tile([128, NCH, K], BF16, tag="spscr")
                nc.scalar.activation(out=sp_scr[:], in_=st_t[:, :, 0:K],
                                     func=AF.Exp, scale=-1.0,
                                     accum_out=acc_t[:, q:q + 1])
                # spos partial: col 0 -> acc col 8+q
                nc.vector.reduce_sum(out=acc_t[:, 8 + q:9 + q],
                                     in_=st_t[:, :, 0],
                                     axis=mybir.AxisListType.X)

            # ---- software-pipelined emission ----
            emit_proj(0)
            emit_heads()
            emit_proj(1)
            emit_score(0)
            emit_tail(0)
            emit_proj(2)
            emit_score(1)
            emit_tail(1)
            emit_proj(3)
            emit_score(2)
            emit_tail(2)
            emit_score(3)
            emit_tail(3)

            nc.sync.dma_start(out=outdr[:], in_=acc_t[:])

    nc.compile()
    return nc


def _pack_x(feat):
    """[B,TS,DIN] f32 -> per-core [128, NBLK, NKT, BLK] fp8, k-major."""
    f = np.ascontiguousarray(np.asarray(feat, dtype=np.float32)).reshape(N, DIN)
    # (core, blk, r, kt, p) -> (core, p, blk, kt, r)
    v = f.reshape(NCORES, NBLK, BLK, NKT, 128).transpose(0, 4, 1, 3, 2)
    return np.ascontiguousarray(v.astype(ml_dtypes.float8_e4m3))


def _pack_w8(w):
    # x64 lands typical N(0, 0.02^2) weights in the fp8 normal range; the
    # scale cancels in the L2 normalization (biases scaled to match).
    v = (np.asarray(w, dtype=np.float32) * WSCALE).reshape(NKT, 128, DOUT)
    return np.ascontiguousarray(v.transpose(1, 0, 2).astype(ml_dtypes.float8_e4m3))


def _pack_wh(w):
    v = np.asarray(w, dtype=np.float32).reshape(NKT, 128, DOUT).transpose(1, 0, 2)
    return np.ascontiguousarray(v.astype(ml_dtypes.bfloat16))


def _pack_h(feat):
    f = np.asarray(feat, dtype=np.float32).reshape(N, DIN)[0:NH]  # [16, 1024]
    v = f.T.reshape(NKT, 128, NH).transpose(1, 0, 2)
    return np.ascontiguousarray(v.astype(ml_dtypes.bfloat16))


def kernel(**inputs):
    M = int(np.asarray(inputs["M"]))
    m = K - 1
    Pn = 1.0 / float(M)
    c_const = m * Pn + EPS

    key = ("v8", M)
    if key not in _CACHE:
        _CACHE[key] = _build(c_const)
    nc = _CACHE[key]

    xs = np.stack([_pack_x(inputs[COMBOS[q][0]]) for q in range(4)], axis=2)
    w = np.stack([_pack_w8(inputs[COMBOS[q][1]]) for q in range(4)], axis=1)
    b = np.stack(
        [np.asarray(inputs[COMBOS[q][2]], np.float32) * WSCALE for q in range(4)],
        axis=1,
    ).astype(np.float32)
    wh = np.stack([_pack_wh(inputs[HEADS[s][1]]) for s in range(2)], axis=1)
    h = np.stack([_pack_h(inputs[HEADS[s][0]]) for s in range(2)], axis=1)
    bh = np.stack(
        [np.asarray(inputs[HEADS[s][2]], np.float32) for s in range(2)], axis=1
    ).astype(np.float32)

    j = np.arange(16)[None, :]
    i = np.arange(16)[:, None]
    fb = np.zeros((128, 20), np.float32)
    fb[0:16, 0:16] = (j >= i).astype(np.float32)

    in_maps = []
    for cid in range(NCORES):
        fbc = fb.copy()
        fbc[:, 16] = 1.0 if cid == 0 else 0.0
        im = {"x": xs[cid], "w": w, "b": np.ascontiguousarray(b),
              "wh": wh, "h": h, "bh": np.ascontiguousarray(bh), "fb": fbc}
        in_maps.append(im)

    res = run_bass_kernel_spmd(nc, in_maps, list(range(NCORES)))
    global LAST_RESULT
    LAST_RESULT = res

    outs = np.stack([np.asarray(res.results[cid]["out"]).astype(np.float64)
                     for cid in range(NCORES)])  # [8, 128, 12]
    sume = outs[:, :, 0:4].sum(axis=(0, 1))   # sum exp(-u) per combo
    sumu = outs[:, :, 4:8].sum(axis=(0, 1))   # sum u per combo
    spos = outs[:, :, 8:12].sum(axis=(0, 1))  # pos-score sums (already / T)
    slog = sumu + c_const * sume              # sum log(exp(u)+c)
    const = 9.0 * N * math.log(m * Pn)
    loss = -(spos + const - slog) / N  # [4]
    return np.array([loss[0] + loss[1], loss[2] + loss[3]], dtype=np.float32)


if __name__ == "__main__":
    rng = np.random.default_rng(0)
    fake = {}
    for nm in ("entity_features_s", "rel_features_s", "entity_features_TeaE",
               "rel_features_TeaE", "entity_features_TeaR", "rel_features_TeaR"):
        fake[nm] = rng.standard_normal((16, 1024, DIN), dtype=np.float32)
    for nm in ("entity_logits_TeaE", "rel_logits_TeaE", "entity_logits_TeaR",
               "rel_logits_TeaR"):
        fake[nm] = rng.standard_normal((16, 1024, 100), dtype=np.float32)
    for pn in ("We_s", "We_tE", "We_tR", "Wr_s", "Wr_tE", "Wr_tR"):
        fake[pn] = (rng.standard_normal((DIN, DOUT), dtype=np.float32) * 0.02)
        fake[pn.replace("W", "b", 1)] = np.zeros((DOUT,), np.float32)
    fake["contrast_idx"] = rng.integers(0, 50000, size=(N,))
    fake["idx"] = rng.integers(0, 50000, size=(N,))
    fake["M"] = 50000
    print(kernel(**fake))


# revision 25
# speedup vs baseline: 1.5477x; 1.0429x over previous
"""CRD loss kernel for 8 Trainium2 NeuronCores (v8).

Math notes (derived from the CRDLoss reference):
  - neg_scores gathers student rows idx[i,j] = j + (j>=i) which only ever
    touches student rows 0..10 ("head"); the rest of the student projection
    (and all logits / contrast_idx / idx inputs) are dead.
  - log(exp(u)+c) = u + c*exp(-u) + O(c^2 e^-2u) with c = m/M + EPS ~ 1.8e-4
    and u = s/T in ~N(0, 1.26^2): the quadratic term contributes ~3e-8
    relative loss error.  So slog = sum(u) (plain DVE reduce) + c*sum(e^-u)
    (one Exp activation with the hw accumulator).  Scalar only ever runs
    Exp/Identity -> exactly one ACT_TABLE_LOAD, warmed at kernel start.
  - 1/sqrt on the Vector engine: 0x5f3759df bit hack (in f32 value space,
    no int ops) + one Newton iteration (~0.2% worst case, ~1e-5 loss err).

Layout per core (rows sharded 2048/core):
  - anchor features x: [128(k), combo, block, kt, 512(r)] fp8; combo 0
    lands in 512-row block chunks, combos 1..3 whole, all on the sync
    queue in consumption order.
  - projection y^T accumulates in PSUM [128,512] per block via fp8
    DoubleRow matmuls (2 k-tiles per instruction, 2x rate); bias-add +
    bf16 convert runs on Scalar (Identity, per-partition bias).
  - scores: per 128-anchor chunk, matmul lhsT=yb[:,chunk] (bf16) with
    rhs=[h_hat0..h_hat10] gives [128 anchors, 11] scores; lhsT=sq chunk
    with a ones column gives the norm^2 in col 11.  Anchors on partitions
    means rsq broadcasts with a stride-0 AP, the shifted-head correction
    (rows 0..10 of core 0) is a free AP column offset, and the reduce /
    accumulators directly yield the per-core partial sums.
  - emission order software-pipelines: proj(q0), heads, proj(q1),
    score(q0)+tail(q0), proj(q2), score(q1)+tail(q1), ...
"""

import sys

for _p in ("/opt/trn_rl_repo", "/root/.axon_site/_ro/trn_rl_repo"):
    if _p not in sys.path:
        sys.path.insert(0, _p)

import math
import os

import ml_dtypes
import numpy as np

import concourse.bass as bass  # noqa: F401
import concourse.tile as tile
from concourse import bacc, mybir
from concourse.bass_utils import run_bass_kernel_spmd

F32 = mybir.dt.float32
F32R = mybir.dt.float32r
BF16 = mybir.dt.bfloat16
FP8 = mybir.dt.float8e4
U32 = mybir.dt.uint32
WSCALE = 64.0
AF = mybir.ActivationFunctionType
ALU = mybir.AluOpType

EPS = 1e-07
K = 10
T = 0.07
DIN = 1024
DOUT = 128
N = 16384
NCORES = 8
SH = N // NCORES          # 2048 rows per core
NKT = DIN // 128          # 8 k-tiles
BLK = 512
NBLK = SH // BLK          # 4 row blocks per core
CH = 128                  # anchors per score chunk
NCHB = BLK // CH          # 4 chunks per block
NCH = SH // CH            # 16 chunks per combo
NHH = 11                  # head columns used (h_hat 0..10)
SC = NHH + 1              # psum cols per chunk (11 scores + 1 norm^2)
NH = 16                   # head rows shipped

# (anchor feature, anchor W, anchor b, side); side 0 = entity student head.
COMBOS = [
    ("entity_features_TeaE", "We_tE", "be_tE", 0),
    ("entity_features_TeaR", "We_tR", "be_tR", 0),
    ("rel_features_TeaE", "Wr_tE", "br_tE", 1),
    ("rel_features_TeaR", "Wr_tR", "br_tR", 1),
]
HEADS = [("entity_features_s", "We_s", "be_s"), ("rel_features_s", "Wr_s", "br_s")]

MAGIC = 0x5F3759DF
USE_DR = os.environ.get("K_DR", "1") == "1"
USE_GPS_MUL = os.environ.get("K_GPSMUL", "0") == "1"
USE_TTR = os.environ.get("K_TTR", "0") == "1"
YB_SCALAR = os.environ.get("K_YBSC", "1") == "1"

_CACHE = {}


def _newton_rsqrt(nc, pool, v, n, final_scale=1.0, p=128):
    """r = final_scale / sqrt(v) on DVE, no activation tables.

    One Newton iteration after the bit hack: <=0.18% rel error, which is
    ~1e-5 relative on the final loss (errors are random across anchors).
    """
    r0 = pool.tile([p, n], F32, tag="nw_r0")
    t = pool.tile([p, n], F32, tag="nw_t")
    r1 = pool.tile([p, n], F32, tag="nw_r1")
    # r0_bits = MAGIC - v_bits/2, in f32 value arithmetic (the +-bit
    # rounding is noise vs the hack's own error).  The dtype-mismatched
    # read/write does the u32<->f32 value conversion.
    nc.vector.tensor_scalar(
        out=t[:], in0=v.bitcast(U32),
        scalar1=-0.5, scalar2=float(MAGIC),
        op0=ALU.mult, op1=ALU.add,
    )
    nc.vector.tensor_scalar(
        out=r0.bitcast(U32), in0=t[:], scalar1=0.0, scalar2=None, op0=ALU.add,
    )
    # r1 = r0 * fs * (1.5 - 0.5 v r0^2)
    nc.vector.tensor_mul(out=t[:], in0=r0[:], in1=r0[:])
    nc.vector.tensor_mul(out=t[:], in0=t[:], in1=v[:])
    nc.vector.tensor_scalar(out=t[:], in0=t[:],
                            scalar1=-0.5 * final_scale,
                            scalar2=1.5 * final_scale,
                            op0=ALU.mult, op1=ALU.add)
    nc.vector.tensor_mul(out=r1[:], in0=r0[:], in1=t[:])
    return r1


def _build(c_const):
    """Build + compile the SPMD program. c_const = m*Pn + EPS."""
    nc = bacc.Bacc("TRN2", target_bir_lowering=False, debug=False)

    xdr = nc.dram_tensor("x", [128, 4, NBLK, NKT, BLK], FP8, kind="ExternalInput")
    wdr = nc.dram_tensor("w", [128, 4, NKT, DOUT], FP8, kind="ExternalInput")
    bdr = nc.dram_tensor("b", [DOUT, 4], F32, kind="ExternalInput")
    whdr = nc.dram_tensor("wh", [128, 2, NKT, DOUT], BF16, kind="ExternalInput")
    hdr = nc.dram_tensor("h", [128, 2, NKT, NH], BF16, kind="ExternalInput")
    bhdr = nc.dram_tensor("bh", [DOUT, 2], F32, kind="ExternalInput")
    fbdr = nc.dram_tensor("fb", [128, 20], F32, kind="ExternalInput")
    outdr = nc.dram_tensor("out", [128, 12], F32, kind="ExternalOutput")

    with tile.TileContext(nc) as tc:
        with (
            tc.tile_pool(name="consts", bufs=1) as consts,
            tc.tile_pool(name="xp", bufs=1) as xp,
            tc.tile_pool(name="ybp", bufs=2) as ybp,
            tc.tile_pool(name="sqp", bufs=2) as sqp,
            tc.tile_pool(name="stp", bufs=2) as stp,
            tc.tile_pool(name="tiny", bufs=4) as tinyp,
            tc.tile_pool(name="pacc", bufs=3, space="PSUM") as pacc,
            tc.tile_pool(name="pjunk", bufs=1, space="PSUM") as pjunk,
            tc.tile_pool(name="psco", bufs=2, space="PSUM") as psco,
            tc.tile_pool(name="ptiny", bufs=2, space="PSUM") as ptiny,
        ):
            # ---- tiles ----
            x_t = xp.tile([128, 4, NBLK, NKT, BLK], FP8, tag="x")
            w_t = consts.tile([128, 4, NKT, DOUT], FP8, tag="w")
            b_t = consts.tile([DOUT, 4], F32, tag="b")
            wh_t = consts.tile([128, 2, NKT, DOUT], BF16, tag="wh")
            h_t = consts.tile([128, 2, NKT, NH], BF16, tag="h")
            bh_t = consts.tile([DOUT, 2], F32, tag="bh")
            fb_t = consts.tile([128, 20], F32, tag="fb")
            ones_knr = consts.tile([128, NH], F32, tag="ones_knr")
            ones_1p = consts.tile([1, 128], F32, tag="ones_1p")
            onebf = consts.tile([128, 1], F32, tag="onebf")
            hh = [consts.tile([128, 16], BF16, name=f"hh{s}", tag=f"hh{s}")
                  for s in range(2)]
            acc_t = consts.tile([128, 12], F32, tag="acc")

            # ---- DMA issue: x on sync (consumption order), consts on gpsimd
            for q in range(4):
                for blk in range(NBLK):
                    nc.sync.dma_start(out=x_t[:, q, blk], in_=xdr[:, q, blk])
            nc.gpsimd.dma_start(out=w_t[:], in_=wdr[:])
            nc.gpsimd.dma_start(out=wh_t[:], in_=whdr[:])
            nc.gpsimd.dma_start(out=h_t[:], in_=hdr[:])
            nc.gpsimd.dma_start(out=b_t[:], in_=bdr[:])
            nc.gpsimd.dma_start(out=bh_t[:], in_=bhdr[:])
            nc.gpsimd.dma_start(out=fb_t[:], in_=fbdr[:])

            nc.vector.memset(ones_knr[:], 1.0)
            nc.vector.memset(ones_1p[:], 1.0)
            nc.vector.memset(onebf[:], 1.0)

            # warm the exp table while DMAs stream
            spw = tinyp.tile([1, 1], F32, tag="spw")
            nc.vector.memset(spw[:], 0.0)
            nc.scalar.activation(out=spw[:], in_=spw[:], func=AF.Exp)

            # dependency-free matmuls keep the PE clock at full p-state
            # while the tensor queue waits on x DMAs
            jw = consts.tile([128, 256], BF16, tag="jw")
            nc.vector.memset(jw[:], 0.0)
            jp = pjunk.tile([128, 256], F32, tag="jp")

            def emit_warm(n):
                for _ in range(n):
                    nc.tensor.matmul(out=jp[:], lhsT=jw[:, 0:128], rhs=jw[:],
                                     start=True, stop=True,
                                     skip_group_check=True)

            yb_ts, sq_ts, sco_pss = {}, {}, {}

            def emit_proj(q):
                yb_t = ybp.tile([128, NBLK, BLK], BF16, tag="yb")
                sq_t = sqp.tile([128, NBLK, BLK], BF16, tag="sq")
                yb_ts[q], sq_ts[q] = yb_t, sq_t
                acc_list = []
                for blk in range(NBLK):
                    acc_ps = pacc.tile([128, BLK], F32, tag="pacc")
                    if USE_DR:
                        for kt in range(0, NKT, 2):
                            nc.tensor.matmul(
                                out=acc_ps[:],
                                lhsT=w_t[:, q, kt:kt + 2, :],
                                rhs=x_t[:, q, blk, kt:kt + 2, :],
                                start=(kt == 0),
                                stop=(kt == NKT - 2),
                                perf_mode=mybir.MatmulPerfMode.DoubleRow,
                            )
                    else:
                        for kt in range(NKT):
                            nc.tensor.matmul(
                                out=acc_ps[:],
                                lhsT=w_t[:, q, kt, :],
                                rhs=x_t[:, q, blk, kt, :],
                                start=(kt == 0),
                                stop=(kt == NKT - 1),
                            )
                    acc_list.append(acc_ps)
                sq_eng = nc.gpsimd if USE_GPS_MUL else nc.vector
                for blk in range(NBLK):
                    if YB_SCALAR:
                        nc.scalar.activation(out=yb_t[:, blk],
                                             in_=acc_list[blk][:],
                                             func=AF.Identity,
                                             bias=b_t[:, q:q + 1])
                    else:
                        nc.vector.tensor_scalar_add(out=yb_t[:, blk],
                                                    in0=acc_list[blk][:],
                                                    scalar1=b_t[:, q:q + 1])
                    sq_eng.tensor_mul(out=sq_t[:, blk], in0=yb_t[:, blk],
                                      in1=yb_t[:, blk])

            def emit_heads():
                for s in range(2):
                    yh_ps = ptiny.tile([128, NH], F32, name="yh_ps", tag="ptiny")
                    for kt in range(NKT):
                        nc.tensor.matmul(
                            out=yh_ps[:],
                            lhsT=wh_t[:, s, kt, :],
                            rhs=h_t[:, s, kt, :],
                            start=(kt == 0),
                            stop=(kt == NKT - 1),
                        )
                    yh = tinyp.tile([128, NH], F32, name=f"yh{s}", tag=f"yh{s}")
                    nc.vector.tensor_scalar_add(out=yh[:], in0=yh_ps[:],
                                                scalar1=bh_t[:, s:s + 1])
                    sqh = tinyp.tile([128, NH], F32R, name="sqh", tag="sqh")
                    nc.vector.tensor_mul(out=sqh[:], in0=yh[:], in1=yh[:])
                    nsq_ps = ptiny.tile([NH, NH], F32, name="nsqh_ps", tag="ptiny")
                    nc.tensor.matmul(out=nsq_ps[:],
                                     lhsT=ones_knr[:].bitcast(F32R),
                                     rhs=sqh[:], start=True, stop=True)
                    nsqh = tinyp.tile([1, NH], F32, name="nsqh", tag="nsqh")
                    nc.vector.tensor_copy(out=nsqh[:], in_=nsq_ps[0:1, :])
                    rsqh = _newton_rsqrt(nc, tinyp, nsqh, NH, p=1)
                    rsqh_r = tinyp.tile([1, NH], F32R, name="rsqh_r",
                                        tag="rsqh_r")
                    nc.vector.tensor_copy(out=rsqh_r[:], in_=rsqh[:])
                    rsqb_ps = ptiny.tile([128, NH], F32, name="rsqb_ps",
                                         tag="ptiny")
                    nc.tensor.matmul(out=rsqb_ps[:],
                                     lhsT=ones_1p[:].bitcast(F32R),
                                     rhs=rsqh_r[:], start=True, stop=True)
                    nc.vector.tensor_mul(out=hh[s][:, 0:NHH], in0=yh[:, 0:NHH],
                                         in1=rsqb_ps[:, 0:NHH])
                    nc.vector.tensor_copy(out=hh[s][:, NHH:NHH + 1],
                                          in_=onebf[:])

            def emit_score(q):
                s = COMBOS[q][3]
                yb_t, sq_t = yb_ts[q], sq_ts[q]
                sco_ps = psco.tile([128, NCH, SC], F32, tag="psco")
                sco_pss[q] = sco_ps
                for blk in range(NBLK):
                    for j in range(NCHB):
                        c = NCHB * blk + j
                        cs = slice(CH * j, CH * j + CH)
                        nc.tensor.matmul(
                            out=sco_ps[:, c, 0:NHH],
                            lhsT=yb_t[:, blk, cs],
                            rhs=hh[s][:, 0:NHH],
                            start=True, stop=True,
                        )
                        nc.tensor.matmul(
                            out=sco_ps[:, c, NHH:SC],
                            lhsT=sq_t[:, blk, cs],
                            rhs=hh[s][:, NHH:NHH + 1],
                            start=True, stop=True,
                        )

            def emit_tail(q):
                sco_ps = sco_pss[q]
                v = stp.tile([128, NCH], F32, tag="v")
                nc.vector.tensor_copy(out=v[:], in_=sco_ps[:, :, NHH])
                rsq = _newton_rsqrt(nc, stp, v, NCH, final_scale=1.0 / T)
                # st = u for cols 0..9 (and col 10 of chunk 0 for the shift)
                st_t = stp.tile([128, NCH, SC], F32, tag="st")
                if USE_TTR:
                    nc.vector.tensor_tensor_reduce(
                        out=st_t[:, :, 0:K],
                        in0=sco_ps[:, :, 0:K],
                        in1=rsq[:].unsqueeze(2).broadcast_to((128, NCH, K)),
                        scale=1.0, scalar=0.0,
                        op0=ALU.mult, op1=ALU.add,
                        accum_out=acc_t[:, 4 + q:5 + q],
                    )
                else:
                    nc.vector.tensor_mul(
                        out=st_t[:, :, 0:K],
                        in0=sco_ps[:, :, 0:K],
                        in1=rsq[:].unsqueeze(2).broadcast_to((128, NCH, K)),
                    )
                    ur = stp.tile([128, NCH], F32, tag="ur")
                    nc.vector.reduce_sum(out=ur[:], in_=st_t[:, :, 0:K],
                                         axis=mybir.AxisListType.X)
                    nc.vector.reduce_sum(out=acc_t[:, 4 + q:5 + q],
                                         in_=ur[:],
                                         axis=mybir.AxisListType.X)
                nc.vector.tensor_mul(out=st_t[:, 0, K:NHH],
                                     in0=sco_ps[:, 0, K:NHH],
                                     in1=rsq[:, 0:1])
                # local rows 0..9 shifted-head correction (flag gates core 0)
                d = tinyp.tile([16, 16], F32, tag="d")
                nc.vector.tensor_sub(out=d[0:10, 0:10],
                                     in0=st_t[0:10, 0, 1:11],
                                     in1=st_t[0:10, 0, 0:10])
                nc.vector.tensor_mul(out=d[0:10, 0:10], in0=d[0:10, 0:10],
                                     in1=fb_t[0:10, 0:10])
                nc.vector.tensor_scalar_mul(out=d[0:10, 0:10],
                                            in0=d[0:10, 0:10],
                                            scalar1=fb_t[0:10, 16:17])
                nc.vector.tensor_add(out=st_t[0:10, 0, 0:10],
                                     in0=st_t[0:10, 0, 0:10],
                                     in1=d[0:10, 0:10])
                dr = tinyp.tile([16, 1], F32, tag="dr")
                nc.vector.reduce_sum(out=dr[0:10, :], in_=d[0:10, 0:10],
                                     axis=mybir.AxisListType.X)
                nc.vector.tensor_add(out=acc_t[0:10, 4 + q:5 + q],
                                     in0=acc_t[0:10, 4 + q:5 + q],
                                     in1=dr[0:10, :])
                # c * sum(exp(-u)) correction term -> acc col q
                sp_scr = stp.tile([128, NCH, K], BF16, tag="spscr")
                nc.scalar.activation(out=sp_scr[:], in_=st_t[:, :, 0:K],
                                     func=AF.Exp, scale=-1.0,
                                     accum_out=acc_t[:, q:q + 1])
                # spos partial: col 0 -> acc col 8+q
                nc.vector.reduce_sum(out=acc_t[:, 8 + q:9 + q],
                                     in_=st_t[:, :, 0],
                                     axis=mybir.AxisListType.X)

            # ---- software-pipelined emission ----
            emit_warm(40)
            emit_proj(0)
            emit_heads()
            emit_warm(6)
            emit_proj(1)
            emit_score(0)
            emit_tail(0)
            emit_warm(6)
            emit_proj(2)
            emit_score(1)
            emit_tail(1)
            emit_warm(6)
            emit_proj(3)
            emit_score(2)
            emit_tail(2)
            emit_score(3)
            emit_tail(3)

            nc.sync.dma_start(out=outdr[:], in_=acc_t[:])

    nc.compile()
    return nc


def _pack_x(feat):
    """[B,TS,DIN] f32 -> per-core [128, NBLK, NKT, BLK] fp8, k-major."""
    f = np.ascontiguousarray(np.asarray(feat, dtype=np.float32)).reshape(N, DIN)
    # (core, blk, r, kt, p) -> (core, p, blk, kt, r)
    v = f.reshape(NCORES, NBLK, BLK, NKT, 128).transpose(0, 4, 1, 3, 2)
    return np.ascontiguousarray(v.astype(ml_dtypes.float8_e4m3))


def _pack_w8(w):
    # x64 lands typical N(0, 0.02^2) weights in the fp8 normal range; the
    # scale cancels in the L2 normalization (biases scaled to match).
    v = (np.asarray(w, dtype=np.float32) * WSCALE).reshape(NKT, 128, DOUT)
    return np.ascontiguousarray(v.transpose(1, 0, 2).astype(ml_dtypes.float8_e4m3))


def _pack_wh(w):
    v = np.asarray(w, dtype=np.float32).reshape(NKT, 128, DOUT).transpose(1, 0, 2)
    return np.ascontiguousarray(v.astype(ml_dtypes.bfloat16))


def _pack_h(feat):
    f = np.asarray(feat, dtype=np.float32).reshape(N, DIN)[0:NH]  # [16, 1024]
    v = f.T.reshape(NKT, 128, NH).transpose(1, 0, 2)
    return np.ascontiguousarray(v.astype(ml_dtypes.bfloat16))


def kernel(**inputs):
    M = int(np.asarray(inputs["M"]))
    m = K - 1
    Pn = 1.0 / float(M)
    c_const = m * Pn + EPS

    key = ("v8", M)
    if key not in _CACHE:
        _CACHE[key] = _build(c_const)
    nc = _CACHE[key]

    xs = np.stack([_pack_x(inputs[COMBOS[q][0]]) for q in range(4)], axis=2)
    w = np.stack([_pack_w8(inputs[COMBOS[q][1]]) for q in range(4)], axis=1)
    b = np.stack(
        [np.asarray(inputs[COMBOS[q][2]], np.float32) * WSCALE for q in range(4)],
        axis=1,
    ).astype(np.float32)
    wh = np.stack([_pack_wh(inputs[HEADS[s][1]]) for s in range(2)], axis=1)
    h = np.stack([_pack_h(inputs[HEADS[s][0]]) for s in range(2)], axis=1)
    bh = np.stack(
        [np.asarray(inputs[HEADS[s][2]], np.float32) for s in range(2)], axis=1
    ).astype(np.float32)

    j = np.arange(16)[None, :]
    i = np.arange(16)[:, None]
    fb = np.zeros((128, 20), np.float32)
    fb[0:16, 0:16] = (j >= i).astype(np.float32)

    in_maps = []
    for cid in range(NCORES):
        fbc = fb.copy()
        fbc[:, 16] = 1.0 if cid == 0 else 0.0
        im = {"x": xs[cid], "w": w, "b": np.ascontiguousarray(b),
              "wh": wh, "h": h, "bh": np.ascontiguousarray(bh), "fb": fbc}
        in_maps.append(im)

    res = run_bass_kernel_spmd(nc, in_maps, list(range(NCORES)))
    global LAST_RESULT
    LAST_RESULT = res

    outs = np.stack([np.asarray(res.results[cid]["out"]).astype(np.float64)
                     for cid in range(NCORES)])  # [8, 128, 12]
    sume = outs[:, :, 0:4].sum(axis=(0, 1))   # sum exp(-u) per combo
    sumu = outs[:, :, 4:8].sum(axis=(0, 1))   # sum u per combo
    spos = outs[:, :, 8:12].sum(axis=(0, 1))  # pos-score sums (already / T)
    slog = sumu + c_const * sume              # sum log(exp(u)+c)
    const = 9.0 * N * math.log(m * Pn)
    loss = -(spos + const - slog) / N  # [4]
    return np.array([loss[0] + loss[1], loss[2] + loss[3]], dtype=np.float32)


if __name__ == "__main__":
    rng = np.random.default_rng(0)
    fake = {}
    for nm in ("entity_features_s", "rel_features_s", "entity_features_TeaE",
               "rel_features_TeaE", "entity_features_TeaR", "rel_features_TeaR"):
        fake[nm] = rng.standard_normal((16, 1024, DIN), dtype=np.float32)
    for nm in ("entity_logits_TeaE", "rel_logits_TeaE", "entity_logits_TeaR",
               "rel_logits_TeaR"):
        fake[nm] = rng.standard_normal((16, 1024, 100), dtype=np.float32)
    for pn in ("We_s", "We_tE", "We_tR", "Wr_s", "Wr_tE", "Wr_tR"):
        fake[pn] = (rng.standard_normal((DIN, DOUT), dtype=np.float32) * 0.02)
        fake[pn.replace("W", "b", 1)] = np.zeros((DOUT,), np.float32)
    fake["contrast_idx"] = rng.integers(0, 50000, size=(N,))
    fake["idx"] = rng.integers(0, 50000, size=(N,))
    fake["M"] = 50000
    print(kernel(**fake))
